# revision 8
# baseline (speedup 1.0000x reference)
"""Causal self-attention (B=4, T=2048, C=1024, 16 heads) on 8 trn2 NeuronCores.

Sharding: core i handles batch b=i//2 and head-half hh=i%2 (8 of 16 heads).
Each core computes its 8 heads' attention output projected through its slice
of W_proj rows (a partial sum of y); host adds the two head-half partials per
batch and transposes back.

Layout strategy (all on-chip matmuls run in float32r at full PE rate):
  - host pre-transposes x[b] -> xT [C, T] so no on-chip transposes are needed
  - qk^T = W_qk.T @ x (via lhsT=W_qk chunks, rhs=xT chunks): [qk_cols, T]
  - V natural [T, vcols] (via lhsT=xT chunk, rhs=W_v), with a ones column
    per head so the PV matmul also produces the softmax denominator
  - S^T[tk, tq] = K_h @ Q_h^T via lhsT=K^T cols, rhs=Q^T (two heads packed
    into the 128-row PE array with tile_position row groups)
  - causal mask added in PSUM by an identity-weight matmul of a -1e30 mask
  - P^T = exp(S^T/8) on ScalarE (masked entries underflow to exactly 0)
  - O^T[65, tq] accumulates lhsT=V_ext[tk,65], rhs=P^T; row 64 = sum(exp)
  - normalize: reciprocal of row 64 on DVE, gpsimd partition_broadcast,
    multiply on DVE
  - y^T = W_proj.T @ attn_out^T accumulated over head pairs; bias per
    partition; DMA out y^T [C, T]
"""

import sys

sys.path.insert(0, "/opt/trn_rl_repo")

import numpy as np

B, T, C = 4, 2048, 1024
NHEAD_GLOBAL = 16
D = 64
H = 8                    # local heads per core
HP = H // 2              # head pairs
NB = 4                   # tq blocks
BLK = T // NB            # 512
CCH = C // 128           # 8 contraction chunks
TCH = T // 128           # 16 tk chunks
NEG = -1.0e30

MASK_W = [256, 256, 384, 512]     # mask matmul widths per diagonal pos d
SCORE_C0 = [0, 128, 256, 256]     # scores matmul col start per diagonal pos d

_CACHE = {}


def _build_nc():
    import concourse.bass as bass  # noqa: F401
    import concourse.mybir as mybir
    import concourse.tile as tile
    from concourse import bacc

    f32 = mybir.dt.float32
    fr = mybir.dt.float32r

    nc = bacc.Bacc("TRN2", target_bir_lowering=False, debug=False)

    xT = nc.dram_tensor("xT", [C, T], fr, kind="ExternalInput").ap()
    wqk = nc.dram_tensor("wqk", [C, 1024], fr, kind="ExternalInput").ap()
    wv = nc.dram_tensor("wv", [C, 512], fr, kind="ExternalInput").ap()
    wpr = nc.dram_tensor("wproj", [512, C], fr, kind="ExternalInput").ap()
    bqk = nc.dram_tensor("bqk", [128, 8], f32, kind="ExternalInput").ap()
    bpr = nc.dram_tensor("bproj", [128, 8], f32, kind="ExternalInput").ap()
    msk = nc.dram_tensor("masks", [4, 128, 512], fr, kind="ExternalInput").ap()
    idn = nc.dram_tensor("ident", [128, 128], fr, kind="ExternalInput").ap()
    yT = nc.dram_tensor("yT", [C, T], f32, kind="ExternalOutput").ap()

    Exp = mybir.ActivationFunctionType.Exp
    Mult = mybir.AluOpType.mult

    with tile.TileContext(nc) as tc:
        with (
            tc.tile_pool(name="const", bufs=1) as cpool,
            tc.tile_pool(name="kv", bufs=1) as kvpool,
            tc.tile_pool(name="xt", bufs=11) as xtpool,
            tc.tile_pool(name="qt", bufs=1) as qtpool,
            tc.tile_pool(name="pt", bufs=3) as ptpool,
            tc.tile_pool(name="ot", bufs=1) as otpool,
            tc.tile_pool(name="ysb", bufs=2) as ypool,
            tc.tile_pool(name="rcp", bufs=2) as rpool,
            tc.tile_pool(name="pa_ps", bufs=2, space="PSUM") as papool,
            tc.tile_pool(name="st_ps", bufs=4, space="PSUM") as stpool,
            tc.tile_pool(name="o_ps", bufs=2, space="PSUM") as opool,
        ):
            # ---- constants / weights resident in SBUF ----
            wqk_sb = []
            for c in range(CCH):
                t_ = cpool.tile([128, 1024], fr, tag=f"wqk{c}", name=f"wqk{c}")
                nc.sync.dma_start(t_[:], wqk[c * 128 : (c + 1) * 128, :])
                wqk_sb.append(t_)
            wv_sb = []
            for c in range(CCH):
                t_ = cpool.tile([128, 512], fr, tag=f"wv{c}", name=f"wv{c}")
                nc.sync.dma_start(t_[:], wv[c * 128 : (c + 1) * 128, :])
                wv_sb.append(t_)
            wpr_sb = []
            for c in range(4):
                t_ = cpool.tile([128, 1024], fr, tag=f"wpr{c}", name=f"wpr{c}")
                nc.sync.dma_start(t_[:], wpr[c * 128 : (c + 1) * 128, :])
                wpr_sb.append(t_)
            mask_sb = []
            for d in range(4):
                t_ = cpool.tile([128, MASK_W[d]], fr, tag=f"msk{d}", name=f"msk{d}")
                nc.sync.dma_start(t_[:], msk[d, :, : MASK_W[d]])
                mask_sb.append(t_)
            ident_sb = cpool.tile([128, 128], fr, tag="ident", name="ident")
            nc.sync.dma_start(ident_sb[:], idn[:, :])
            bqk_sb = cpool.tile([128, 8], f32, tag="bqk", name="bqk")
            nc.sync.dma_start(bqk_sb[:], bqk[:, :])
            bpr_sb = cpool.tile([128, 8], f32, tag="bpr", name="bpr")
            nc.sync.dma_start(bpr_sb[:], bpr[:, :])
            ones_sb = cpool.tile([128, 1], f32, tag="ones", name="ones")
            nc.vector.memset(ones_sb[:], 1.0)

            # persistent K^T tiles per (head-pair, block) and V tiles per tk chunk
            kT = [[None] * NB for _ in range(HP)]
            vt = [None] * TCH

            for j in range(NB):
                # ---------------- phase A: QKV for block j ----------------
                xt = []
                for c in range(CCH):
                    t_ = xtpool.tile([128, BLK], fr, tag="xt", name="xt")
                    nc.sync.dma_start(
                        t_[:], xT[c * 128 : (c + 1) * 128, j * BLK : (j + 1) * BLK]
                    )
                    xt.append(t_)

                qT = []
                for t in range(8):
                    ps = papool.tile([128, BLK], f32, tag="pa", name="pa")
                    for c in range(CCH):
                        nc.tensor.matmul(
                            ps[:],
                            lhsT=wqk_sb[c][:, t * 128 : (t + 1) * 128],
                            rhs=xt[c][:],
                            start=(c == 0),
                            stop=(c == CCH - 1),
                        )
                    if t < 4:
                        dst = qtpool.tile([128, BLK], fr, tag=f"qT{t}", name=f"qT{t}")
                        qT.append(dst)
                    else:
                        dst = kvpool.tile(
                            [128, BLK], fr, tag=f"kT{t - 4}_{j}", name=f"kT{t - 4}_{j}"
                        )
                        kT[t - 4][j] = dst
                    nc.vector.tensor_scalar_add(dst[:], ps[:], bqk_sb[:, t : t + 1])

                for tcl in range(4):
                    tci = 4 * j + tcl
                    ps = papool.tile([128, BLK], f32, tag="pa", name="pa")
                    for c in range(CCH):
                        nc.tensor.matmul(
                            ps[:],
                            lhsT=xt[c][:, tcl * 128 : (tcl + 1) * 128],
                            rhs=wv_sb[c][:],
                            start=(c == 0),
                            stop=(c == CCH - 1),
                        )
                    v_ = kvpool.tile([128, H, 65], fr, tag=f"v{tci}", name=f"v{tci}")
                    vt[tci] = v_
                    nc.vector.tensor_copy(
                        v_[:, :, 64], ones_sb[:, 0:1].to_broadcast([128, H])
                    )
                    nc.vector.tensor_copy(
                        v_[:, :, :64], ps[:].rearrange("p (h d) -> p h d", d=64)
                    )

                # ---------------- phase B: attention for block j ----------------
                nchunks = 4 * j + 4
                oT = [
                    otpool.tile([128, BLK], fr, tag=f"oT{hp}", name=f"oT{hp}")
                    for hp in range(HP)
                ]
                for hp in range(HP):
                    o_ps = [
                        opool.tile([65, BLK], f32, tag="o", name="o_ps")
                        for _ in range(2)
                    ]
                    prev_pt = None
                    for tci in range(nchunks):
                        d = tci - 4 * j
                        cur_pt = []
                        for s in range(2):
                            h = 2 * hp + s
                            st = stpool.tile([128, BLK], f32, tag="st", name="st")
                            kslc = kT[hp][tci // 4][
                                64 * s : 64 * s + 64,
                                (tci % 4) * 128 : (tci % 4 + 1) * 128,
                            ]
                            if d < 0:
                                nc.tensor.matmul(
                                    st[:],
                                    lhsT=kslc,
                                    rhs=qT[hp][64 * s : 64 * s + 64, :],
                                    start=True,
                                    stop=True,
                                    tile_position=(64 * s, 0),
                                )
                            else:
                                nc.tensor.matmul(
                                    st[:, : MASK_W[d]],
                                    lhsT=ident_sb[:],
                                    rhs=mask_sb[d][:],
                                    start=True,
                                    stop=False,
                                )
                                c0 = SCORE_C0[d]
                                nc.tensor.matmul(
                                    st[:, c0:],
                                    lhsT=kslc,
                                    rhs=qT[hp][64 * s : 64 * s + 64, c0:],
                                    start=False,
                                    stop=True,
                                    tile_position=(64 * s, 0),
                                )
                            pt = ptpool.tile([128, BLK], fr, tag="pT", name="pT")
                            nc.scalar.activation(pt[:], st[:], Exp, scale=0.125)
                            cur_pt.append((h, s, pt))
                        if prev_pt is not None:
                            for (h_, s_, pt_), tcp in prev_pt:
                                nc.tensor.matmul(
                                    o_ps[s_][:],
                                    lhsT=vt[tcp][:, h_, :],
                                    rhs=pt_[:],
                                    start=(tcp == 0),
                                    stop=(tcp == nchunks - 1),
                                )
                        prev_pt = [(x, tci) for x in cur_pt]
                    for (h_, s_, pt_), tcp in prev_pt:
                        nc.tensor.matmul(
                            o_ps[s_][:],
                            lhsT=vt[tcp][:, h_, :],
                            rhs=pt_[:],
                            start=(tcp == 0),
                            stop=(tcp == nchunks - 1),
                        )
                    for s in range(2):
                        rc = rpool.tile([1, BLK], f32, tag="rcp", name="rcp")
                        nc.vector.reciprocal(rc[:], o_ps[s][64:65, :])
                        bc = rpool.tile([64, BLK], f32, tag="bc", name="bc")
                        nc.gpsimd.partition_broadcast(bc[:], rc[:])
                        nc.vector.tensor_tensor(
                            oT[hp][64 * s : 64 * s + 64, :],
                            o_ps[s][:64, :],
                            bc[:],
                            Mult,
                        )

                # ---------------- phase C: output projection ----------------
                for t in range(8):
                    ps = papool.tile([128, BLK], f32, tag="pa", name="pa")
                    for cp in range(4):
                        nc.tensor.matmul(
                            ps[:],
                            lhsT=wpr_sb[cp][:, t * 128 : (t + 1) * 128],
                            rhs=oT[cp][:],
                            start=(cp == 0),
                            stop=(cp == 3),
                        )
                    ysb = ypool.tile([128, BLK], f32, tag="y", name="ysb")
                    nc.vector.tensor_scalar_add(ysb[:], ps[:], bpr_sb[:, t : t + 1])
                    nc.sync.dma_start(
                        yT[t * 128 : (t + 1) * 128, j * BLK : (j + 1) * BLK], ysb[:]
                    )

    nc.compile()
    return nc


def _host_inputs(x, W_attn, b_attn, W_proj, b_proj):
    """Build the 8 per-core input maps."""
    x = np.asarray(x, dtype=np.float32)
    W_attn = np.asarray(W_attn, dtype=np.float32)
    b_attn = np.asarray(b_attn, dtype=np.float32)
    W_proj = np.asarray(W_proj, dtype=np.float32)
    b_proj = np.asarray(b_proj, dtype=np.float32)

    masks = np.zeros((4, 128, 512), dtype=np.float32)
    p = np.arange(128)[:, None]
    f = np.arange(512)[None, :]
    for d in range(4):
        masks[d] = np.where(f >= 128 * d + p, 0.0, NEG).astype(np.float32)
    ident = np.eye(128, dtype=np.float32)

    in_maps = []
    for core in range(8):
        b = core // 2
        hh = core % 2
        cs = hh * 512
        wq = W_attn[:, cs : cs + 512]
        wk = W_attn[:, C + cs : C + cs + 512]
        wv_ = W_attn[:, 2 * C + cs : 2 * C + cs + 512]
        bq = b_attn[cs : cs + 512]
        bk = b_attn[C + cs : C + cs + 512]
        bv = b_attn[2 * C + cs : 2 * C + cs + 512]
        wpr_ = W_proj[cs : cs + 512, :]
        bpr_eff = bv @ wpr_ + (b_proj if hh == 0 else 0.0)
        in_maps.append(
            {
                "xT": np.ascontiguousarray(x[b].T),
                "wqk": np.ascontiguousarray(np.concatenate([wq, wk], axis=1)),
                "wv": np.ascontiguousarray(wv_),
                "wproj": np.ascontiguousarray(wpr_),
                "bqk": np.ascontiguousarray(
                    np.concatenate([bq, bk]).reshape(8, 128).T
                ),
                "bproj": np.ascontiguousarray(
                    bpr_eff.astype(np.float32).reshape(8, 128).T
                ),
                "masks": masks,
                "ident": ident,
            }
        )
    return in_maps


def run(x, W_attn, b_attn, W_proj, b_proj, trace=False):
    from concourse.bass_utils import run_bass_kernel_spmd

    if "nc" not in _CACHE:
        _CACHE["nc"] = _build_nc()
    nc = _CACHE["nc"]
    in_maps = _host_inputs(x, W_attn, b_attn, W_proj, b_proj)
    res = run_bass_kernel_spmd(nc, in_maps, core_ids=list(range(8)), trace=False)
    y = np.empty((B, T, C), dtype=np.float32)
    for b in range(B):
        y[b] = (res.results[2 * b]["yT"] + res.results[2 * b + 1]["yT"]).T
    return y, res


def kernel(x, W_attn, b_attn, W_proj, b_proj):
    y, _ = run(x, W_attn, b_attn, W_proj, b_proj, trace=False)
    return y


def make_timed_runner(in_maps=None, nc=None):
    """Build a non-donating jitted SPMD callable with device-resident inputs.

    Returns fn(n) -> wall seconds to execute the kernel n times back-to-back
    (async dispatch, single block at the end). Differential timing
    (wall(n) - wall(1)) / (n - 1) estimates per-execution device time.
    """
    import jax
    import numpy as _np
    import concourse.mybir as mybir
    from concourse import bass2jax
    from jax.experimental.shard_map import shard_map
    from jax.sharding import Mesh, PartitionSpec, NamedSharding

    if nc is None:
        if "nc" not in _CACHE:
            _CACHE["nc"] = _build_nc()
        nc = _CACHE["nc"]

    bass2jax.install_neuronx_cc_hook()
    n_cores = 8

    partition_name = nc.partition_id_tensor.name if nc.partition_id_tensor else None
    in_names, out_names, out_avals, zero_outs = [], [], [], []
    for alloc in nc.m.functions[0].allocations:
        if not isinstance(alloc, mybir.MemoryLocationSet):
            continue
        name = alloc.memorylocations[0].name
        if alloc.kind == "ExternalInput":
            if name != partition_name:
                in_names.append(name)
        elif alloc.kind == "ExternalOutput":
            out_names.append(name)
            shape = tuple(alloc.tensor_shape)
            dtype = mybir.dt.np(alloc.dtype)
            out_avals.append(jax.core.ShapedArray(shape, dtype))
            zero_outs.append(_np.zeros(shape, dtype))
    n_params = len(in_names)
    all_names = in_names + out_names
    if partition_name is not None:
        all_names = all_names + [partition_name]

    def _body(*args):
        operands = list(args)
        if partition_name is not None:
            operands.append(bass2jax.partition_id_tensor())
        outs = bass2jax._bass_exec_p.bind(
            *operands,
            out_avals=tuple(out_avals),
            in_names=tuple(all_names),
            out_names=tuple(out_names),
            lowering_input_output_aliases=(),
            sim_require_finite=True,
            sim_require_nnan=True,
            nc=nc,
        )
        return tuple(outs)

    devices = jax.devices()[:n_cores]
    mesh = Mesh(_np.asarray(devices), ("core",))
    spec = PartitionSpec("core")
    sharded = jax.jit(
        shard_map(
            _body,
            mesh=mesh,
            in_specs=(spec,) * (n_params + len(out_names)),
            out_specs=(spec,) * len(out_names),
            check_rep=False,
        ),
        keep_unused=True,
    )
    sh = NamedSharding(mesh, spec)
    dev_args = [
        jax.device_put(
            _np.concatenate([_np.asarray(in_maps[c][nm]) for c in range(n_cores)], 0),
            sh,
        )
        for nm in in_names
    ] + [
        jax.device_put(
            _np.zeros((n_cores * z.shape[0], *z.shape[1:]), z.dtype), sh
        )
        for z in zero_outs
    ]

    import time as _time

    def timed(n):
        out = None
        t0 = _time.perf_counter()
        for _ in range(n):
            out = sharded(*dev_args)
        jax.block_until_ready(out)
        return _time.perf_counter() - t0

    return timed


# revision 10
# speedup vs baseline: 1.4229x; 1.4229x over previous
"""Causal self-attention (B=4, T=2048, C=1024, 16 heads) on 8 trn2 NeuronCores.

Sharding: core i handles batch b=i//2 and head-half hh=i%2 (8 of 16 heads).
Each core computes its 8 heads' attention output projected through its slice
of W_proj rows (a partial sum of y); host adds the two head-half partials per
batch and transposes back.

Layout strategy (matmul operands in bf16, fp32 PSUM accumulation):
  - host pre-transposes x[b] -> xT [C, T] so no on-chip transposes are needed
  - qk^T = W_qk.T @ x (via lhsT=W_qk chunks, rhs=xT chunks): [qk_cols, T]
  - V natural [T, vcols] (via lhsT=xT chunk, rhs=W_v), with a ones column
    per head so the PV matmul also produces the softmax denominator
  - S^T[tk, tq] = K_h @ Q_h^T via lhsT=K^T cols, rhs=Q^T (two heads packed
    into the 128-row PE array with tile_position row groups); both heads of
    a pair share one [128, 1024] PSUM tile so exp runs as a single ACT op
  - causal mask added in PSUM by an identity-weight matmul of a -1e30 mask
  - P^T = exp(S^T/8) on ScalarE (masked entries underflow to exactly 0)
  - O^T[65, tq] accumulates lhsT=V_ext[tk,65], rhs=P^T; row 64 = sum(exp)
  - normalize: reciprocal of row 64 on DVE, gpsimd partition_broadcast,
    multiply on DVE
  - y^T = W_proj.T @ attn_out^T accumulated over head pairs; bias per
    partition; DMA out y^T [C, T]
"""

import sys

sys.path.insert(0, "/opt/trn_rl_repo")

import numpy as np
import ml_dtypes

BF16 = ml_dtypes.bfloat16

B, T, C = 4, 2048, 1024
NHEAD_GLOBAL = 16
D = 64
H = 8                    # local heads per core
HP = H // 2              # head pairs
NB = 4                   # tq blocks
BLK = T // NB            # 512
CCH = C // 128           # 8 contraction chunks
TCH = T // 128           # 16 tk chunks
NEG = -1.0e30

MASK_W = [128, 256, 384, 512]     # mask matmul widths per diagonal pos d
SCORE_C0 = [0, 128, 256, 384]     # scores/pv matmul col start per diagonal pos d

_CACHE = {}


def _build_nc():
    import concourse.bass as bass  # noqa: F401
    import concourse.mybir as mybir
    import concourse.tile as tile
    from concourse import bacc

    f32 = mybir.dt.float32
    bf = mybir.dt.bfloat16

    nc = bacc.Bacc("TRN2", target_bir_lowering=False, debug=False)

    xT = nc.dram_tensor("xT", [C, T], bf, kind="ExternalInput").ap()
    wqk = nc.dram_tensor("wqk", [C, 1024], bf, kind="ExternalInput").ap()
    wv = nc.dram_tensor("wv", [C, 512], bf, kind="ExternalInput").ap()
    wpr = nc.dram_tensor("wproj", [512, C], bf, kind="ExternalInput").ap()
    bqk = nc.dram_tensor("bqk", [128, 8], f32, kind="ExternalInput").ap()
    bpr = nc.dram_tensor("bproj", [128, 8], f32, kind="ExternalInput").ap()
    msk = nc.dram_tensor("masks", [4, 128, 512], bf, kind="ExternalInput").ap()
    idn = nc.dram_tensor("ident", [128, 128], bf, kind="ExternalInput").ap()
    yT = nc.dram_tensor("yT", [C, T], f32, kind="ExternalOutput").ap()

    Exp = mybir.ActivationFunctionType.Exp
    Mult = mybir.AluOpType.mult

    with tile.TileContext(nc) as tc:
        with (
            tc.tile_pool(name="const", bufs=1) as cpool,
            tc.tile_pool(name="kv", bufs=1) as kvpool,
            tc.tile_pool(name="xt", bufs=16) as xtpool,
            tc.tile_pool(name="qt", bufs=2) as qtpool,
            tc.tile_pool(name="pt", bufs=6) as ptpool,
            tc.tile_pool(name="ot", bufs=2) as otpool,
            tc.tile_pool(name="ysb", bufs=2) as ypool,
            tc.tile_pool(name="rcp", bufs=2) as rpool,
            tc.tile_pool(name="pa_ps", bufs=2, space="PSUM") as papool,
            tc.tile_pool(name="st_ps", bufs=2, space="PSUM") as stpool,
            tc.tile_pool(name="o_ps", bufs=2, space="PSUM") as opool,
        ):
            # ---- constants / weights resident in SBUF ----
            wqk_sb = []
            for c in range(CCH):
                t_ = cpool.tile([128, 1024], bf, tag=f"wqk{c}", name=f"wqk{c}")
                nc.sync.dma_start(t_[:], wqk[c * 128 : (c + 1) * 128, :])
                wqk_sb.append(t_)
            wv_sb = []
            for c in range(CCH):
                t_ = cpool.tile([128, 512], bf, tag=f"wv{c}", name=f"wv{c}")
                nc.sync.dma_start(t_[:], wv[c * 128 : (c + 1) * 128, :])
                wv_sb.append(t_)
            wpr_sb = []
            for c in range(4):
                t_ = cpool.tile([128, 1024], bf, tag=f"wpr{c}", name=f"wpr{c}")
                nc.sync.dma_start(t_[:], wpr[c * 128 : (c + 1) * 128, :])
                wpr_sb.append(t_)
            mask_sb = []
            for d in range(4):
                t_ = cpool.tile([128, MASK_W[d]], bf, tag=f"msk{d}", name=f"msk{d}")
                nc.sync.dma_start(t_[:], msk[d, :, : MASK_W[d]])
                mask_sb.append(t_)
            ident_sb = cpool.tile([128, 128], bf, tag="ident", name="ident")
            nc.sync.dma_start(ident_sb[:], idn[:, :])
            bqk_sb = cpool.tile([128, 8], f32, tag="bqk", name="bqk")
            nc.sync.dma_start(bqk_sb[:], bqk[:, :])
            bpr_sb = cpool.tile([128, 8], f32, tag="bpr", name="bpr")
            nc.sync.dma_start(bpr_sb[:], bpr[:, :])
            ones_sb = cpool.tile([128, 1], f32, tag="ones", name="ones")
            nc.vector.memset(ones_sb[:], 1.0)

            # persistent K^T tiles per (head-pair, block) and V tiles per tk chunk
            kT = [[None] * NB for _ in range(HP)]
            vt = [None] * TCH

            for j in range(NB):
                # ---------------- phase A: QKV for block j ----------------
                xt = []
                for c in range(CCH):
                    t_ = xtpool.tile([128, BLK], bf, tag="xt", name="xt")
                    nc.sync.dma_start(
                        t_[:], xT[c * 128 : (c + 1) * 128, j * BLK : (j + 1) * BLK]
                    )
                    xt.append(t_)

                qT = []
                for t in range(8):
                    ps = papool.tile([128, BLK], f32, tag="pa", name="pa")
                    for c in range(CCH):
                        nc.tensor.matmul(
                            ps[:],
                            lhsT=wqk_sb[c][:, t * 128 : (t + 1) * 128],
                            rhs=xt[c][:],
                            start=(c == 0),
                            stop=(c == CCH - 1),
                        )
                    if t < 4:
                        dst = qtpool.tile([128, BLK], bf, tag=f"qT{t}", name=f"qT{t}")
                        qT.append(dst)
                    else:
                        dst = kvpool.tile(
                            [128, BLK], bf, tag=f"kT{t - 4}_{j}", name=f"kT{t - 4}_{j}"
                        )
                        kT[t - 4][j] = dst
                    nc.vector.tensor_scalar_add(dst[:], ps[:], bqk_sb[:, t : t + 1])

                for tcl in range(4):
                    tci = 4 * j + tcl
                    ps = papool.tile([128, BLK], f32, tag="pa", name="pa")
                    for c in range(CCH):
                        nc.tensor.matmul(
                            ps[:],
                            lhsT=xt[c][:, tcl * 128 : (tcl + 1) * 128],
                            rhs=wv_sb[c][:],
                            start=(c == 0),
                            stop=(c == CCH - 1),
                        )
                    v_ = kvpool.tile([128, H, 65], bf, tag=f"v{tci}", name=f"v{tci}")
                    vt[tci] = v_
                    nc.vector.tensor_copy(
                        v_[:, :, 64], ones_sb[:, 0:1].to_broadcast([128, H])
                    )
                    nc.vector.tensor_copy(
                        v_[:, :, :64], ps[:].rearrange("p (h d) -> p h d", d=64)
                    )

                # ---------------- phase B: attention for block j ----------------
                nchunks = 4 * j + 4
                oT = [
                    otpool.tile([128, BLK], bf, tag=f"oT{hp}", name=f"oT{hp}")
                    for hp in range(HP)
                ]
                for hp in range(HP):
                    o_ps = [
                        opool.tile([65, BLK], f32, tag="o", name="o_ps")
                        for _ in range(2)
                    ]
                    prev = None
                    for tci in range(nchunks):
                        d = tci - 4 * j
                        c0 = 0 if d < 0 else SCORE_C0[d]
                        st = stpool.tile([128, 2 * BLK], f32, tag="st", name="st")
                        for s in range(2):
                            sb = s * BLK
                            kslc = kT[hp][tci // 4][
                                64 * s : 64 * s + 64,
                                (tci % 4) * 128 : (tci % 4 + 1) * 128,
                            ]
                            if d >= 0:
                                nc.tensor.matmul(
                                    st[:, sb : sb + MASK_W[d]],
                                    lhsT=ident_sb[:],
                                    rhs=mask_sb[d][:],
                                    start=True,
                                    stop=False,
                                )
                            nc.tensor.matmul(
                                st[:, sb + c0 : sb + BLK],
                                lhsT=kslc,
                                rhs=qT[hp][64 * s : 64 * s + 64, c0:],
                                start=(d < 0),
                                stop=True,
                                tile_position=(64 * s, 0),
                            )
                        pt = ptpool.tile([128, 2 * BLK], bf, tag="pT", name="pT")
                        nc.scalar.activation(pt[:], st[:], Exp, scale=0.125)
                        if prev is not None:
                            pt_p, tcp, c0p = prev
                            for s in range(2):
                                nc.tensor.matmul(
                                    o_ps[s][:, c0p:],
                                    lhsT=vt[tcp][:, 2 * hp + s, :],
                                    rhs=pt_p[:, s * BLK + c0p : (s + 1) * BLK],
                                    start=(tcp == 0),
                                    stop=(tcp == nchunks - 1),
                                )
                        prev = (pt, tci, c0)
                    pt_p, tcp, c0p = prev
                    for s in range(2):
                        nc.tensor.matmul(
                            o_ps[s][:, c0p:],
                            lhsT=vt[tcp][:, 2 * hp + s, :],
                            rhs=pt_p[:, s * BLK + c0p : (s + 1) * BLK],
                            start=(tcp == 0),
                            stop=(tcp == nchunks - 1),
                        )
                    for s in range(2):
                        rc = rpool.tile([1, BLK], f32, tag="rcp", name="rcp")
                        nc.vector.reciprocal(rc[:], o_ps[s][64:65, :])
                        bc = rpool.tile([64, BLK], f32, tag="bc", name="bc")
                        nc.gpsimd.partition_broadcast(bc[:], rc[:])
                        nc.vector.tensor_tensor(
                            oT[hp][64 * s : 64 * s + 64, :],
                            o_ps[s][:64, :],
                            bc[:],
                            Mult,
                        )

                # ---------------- phase C: output projection ----------------
                for t in range(8):
                    ps = papool.tile([128, BLK], f32, tag="pa", name="pa")
                    for cp in range(4):
                        nc.tensor.matmul(
                            ps[:],
                            lhsT=wpr_sb[cp][:, t * 128 : (t + 1) * 128],
                            rhs=oT[cp][:],
                            start=(cp == 0),
                            stop=(cp == 3),
                        )
                    ysb = ypool.tile([128, BLK], f32, tag="y", name="ysb")
                    nc.vector.tensor_scalar_add(ysb[:], ps[:], bpr_sb[:, t : t + 1])
                    nc.sync.dma_start(
                        yT[t * 128 : (t + 1) * 128, j * BLK : (j + 1) * BLK], ysb[:]
                    )

    nc.compile()
    return nc


def _host_inputs(x, W_attn, b_attn, W_proj, b_proj):
    """Build the 8 per-core input maps."""
    x = np.asarray(x, dtype=np.float32)
    W_attn = np.asarray(W_attn, dtype=np.float32)
    b_attn = np.asarray(b_attn, dtype=np.float32)
    W_proj = np.asarray(W_proj, dtype=np.float32)
    b_proj = np.asarray(b_proj, dtype=np.float32)

    masks = np.zeros((4, 128, 512), dtype=np.float32)
    p = np.arange(128)[:, None]
    f = np.arange(512)[None, :]
    for d in range(4):
        masks[d] = np.where(f >= 128 * d + p, 0.0, NEG).astype(np.float32)
    masks = masks.astype(BF16)
    ident = np.eye(128, dtype=np.float32).astype(BF16)

    in_maps = []
    for core in range(8):
        b = core // 2
        hh = core % 2
        cs = hh * 512
        wq = W_attn[:, cs : cs + 512]
        wk = W_attn[:, C + cs : C + cs + 512]
        wv_ = W_attn[:, 2 * C + cs : 2 * C + cs + 512]
        bq = b_attn[cs : cs + 512]
        bk = b_attn[C + cs : C + cs + 512]
        bv = b_attn[2 * C + cs : 2 * C + cs + 512]
        wpr_ = W_proj[cs : cs + 512, :]
        bpr_eff = bv @ wpr_ + (b_proj if hh == 0 else 0.0)
        in_maps.append(
            {
                "xT": np.ascontiguousarray(x[b].T).astype(BF16),
                "wqk": np.ascontiguousarray(
                    np.concatenate([wq, wk], axis=1)
                ).astype(BF16),
                "wv": np.ascontiguousarray(wv_).astype(BF16),
                "wproj": np.ascontiguousarray(wpr_).astype(BF16),
                "bqk": np.ascontiguousarray(
                    np.concatenate([bq, bk]).reshape(8, 128).T
                ),
                "bproj": np.ascontiguousarray(
                    bpr_eff.astype(np.float32).reshape(8, 128).T
                ),
                "masks": masks,
                "ident": ident,
            }
        )
    return in_maps


def run(x, W_attn, b_attn, W_proj, b_proj, trace=False):
    from concourse.bass_utils import run_bass_kernel_spmd

    if "nc" not in _CACHE:
        _CACHE["nc"] = _build_nc()
    nc = _CACHE["nc"]
    in_maps = _host_inputs(x, W_attn, b_attn, W_proj, b_proj)
    res = run_bass_kernel_spmd(nc, in_maps, core_ids=list(range(8)), trace=False)
    y = np.empty((B, T, C), dtype=np.float32)
    for b in range(B):
        y[b] = (res.results[2 * b]["yT"] + res.results[2 * b + 1]["yT"]).T
    return y, res


def kernel(x, W_attn, b_attn, W_proj, b_proj):
    y, _ = run(x, W_attn, b_attn, W_proj, b_proj, trace=False)
    return y


def make_timed_runner(in_maps=None, nc=None):
    """Build a non-donating jitted SPMD callable with device-resident inputs.

    Returns fn(n) -> wall seconds to execute the kernel n times back-to-back
    (async dispatch, single block at the end). Differential timing
    (wall(n) - wall(1)) / (n - 1) estimates per-execution device time.
    """
    import jax
    import numpy as _np
    import concourse.mybir as mybir
    from concourse import bass2jax
    from jax.experimental.shard_map import shard_map
    from jax.sharding import Mesh, PartitionSpec, NamedSharding

    if nc is None:
        if "nc" not in _CACHE:
            _CACHE["nc"] = _build_nc()
        nc = _CACHE["nc"]

    bass2jax.install_neuronx_cc_hook()
    n_cores = 8

    partition_name = nc.partition_id_tensor.name if nc.partition_id_tensor else None
    in_names, out_names, out_avals, zero_outs = [], [], [], []
    for alloc in nc.m.functions[0].allocations:
        if not isinstance(alloc, mybir.MemoryLocationSet):
            continue
        name = alloc.memorylocations[0].name
        if alloc.kind == "ExternalInput":
            if name != partition_name:
                in_names.append(name)
        elif alloc.kind == "ExternalOutput":
            out_names.append(name)
            shape = tuple(alloc.tensor_shape)
            dtype = mybir.dt.np(alloc.dtype)
            out_avals.append(jax.core.ShapedArray(shape, dtype))
            zero_outs.append(_np.zeros(shape, dtype))
    n_params = len(in_names)
    all_names = in_names + out_names
    if partition_name is not None:
        all_names = all_names + [partition_name]

    def _body(*args):
        operands = list(args)
        if partition_name is not None:
            operands.append(bass2jax.partition_id_tensor())
        outs = bass2jax._bass_exec_p.bind(
            *operands,
            out_avals=tuple(out_avals),
            in_names=tuple(all_names),
            out_names=tuple(out_names),
            lowering_input_output_aliases=(),
            sim_require_finite=True,
            sim_require_nnan=True,
            nc=nc,
        )
        return tuple(outs)

    devices = jax.devices()[:n_cores]
    mesh = Mesh(_np.asarray(devices), ("core",))
    spec = PartitionSpec("core")
    sharded = jax.jit(
        shard_map(
            _body,
            mesh=mesh,
            in_specs=(spec,) * (n_params + len(out_names)),
            out_specs=(spec,) * len(out_names),
            check_rep=False,
        ),
        keep_unused=True,
    )
    sh = NamedSharding(mesh, spec)
    dev_args = [
        jax.device_put(
            _np.concatenate([_np.asarray(in_maps[c][nm]) for c in range(n_cores)], 0),
            sh,
        )
        for nm in in_names
    ] + [
        jax.device_put(
            _np.zeros((n_cores * z.shape[0], *z.shape[1:]), z.dtype), sh
        )
        for z in zero_outs
    ]

    import time as _time

    def timed(n):
        out = None
        t0 = _time.perf_counter()
        for _ in range(n):
            out = sharded(*dev_args)
        jax.block_until_ready(out)
        return _time.perf_counter() - t0

    return timed
